# revision 55
# baseline (speedup 1.0000x reference)
"""Trainium2 Bass kernel for batched GCN (2x GCNConv + circular Conv1d).

Math per graph (N=64 nodes, S=96 feats, H=512 hidden, E=512 edges):
    A~       = Dinv (C+I)^T Dinv    (normalized adjacency, transposed)
    u        = X^T A~               ( = (A X)^T,  X = x.T [n, s])
    a1t      = W1 u                 (h on partitions, = (A X W1^T)^T)
    h1t      = relu(a1t)
    z2       = h1 W2^T              (via 4 h-chunk accumulation)
    h2       = A z2   (written shifted+duplicated into P for the conv)
    y        = circular_conv1d(h2, conv_w), emitted transposed [o, (g,l)]

The normalized adjacency A~ is built on the host from edge_index
(vectorized bincount + outer scaling -- standard GNN graph
preprocessing) and shipped per graph as a compact [64, 64] bf16 tile;
all model FLOPs (4 matmul stages + conv) run on device.

Device pipeline: 64 graphs = 32 pairs, pair in partition halves.
  Per pair: u (2 mm) -> a1t (4 mm) -> relu -> z2 (4 mm, emits the
  wrap-padded z2w directly via duplicated W2 columns) -> h2 (4 mm,
  shift-duplicated into P) -> conv (8 mm) -> y evac -> batched DMA.
  Emission is software-pipelined over 4 stage-lags (B1 | relu+B2a |
  B2b | HH+B3) so every engine's in-order queue sees dep-ready work;
  psum = u+z2w+P ring 4 | a1t ring 2 | y ring 2 = 8 banks; psum
  evacuations are balanced Act{u, relu, HH} vs DVE{z2w, yA, yB}
  (min-max over engine-specific fixed+per-col costs), sbuf-only copies
  on GpSimd; inputs split across SP/Act/GpSimd DMA queues; out DMA
  batched 2 pairs/instruction.
"""

import numpy as np
import ml_dtypes

import concourse.bacc as bacc
import concourse.mybir as mybir
import concourse.tile as tile
from concourse.bass_utils import run_bass_kernel_spmd

BF16 = mybir.dt.bfloat16
FP32 = mybir.dt.float32
AF = mybir.ActivationFunctionType
OP = mybir.AluOpType

N_CORES = 8
B, S, N, H, E = 512, 96, 64, 512, 512
G = B // N_CORES          # graphs per core (64)
NPAIR = G // 2            # 32


def build_gcn_kernel(tc, outs, ins, has_b1=False, has_b2=False):
    nc = tc.nc

    axt_d = ins["axt"]      # [64, G*160] bf16  per graph [x^T | A~] on parts 0:64
    cst_d = ins["cst"]      # [128, Wc] bf16 packed consts
    y_d = outs["y"]         # [4, 128, 32, 192] bf16

    from contextlib import ExitStack
    ctx = ExitStack()
    const = ctx.enter_context(tc.tile_pool(name="const", bufs=1))
    sb = ctx.enter_context(tc.tile_pool(name="sb", bufs=1))
    psU = ctx.enter_context(tc.tile_pool(name="psU", bufs=4, space="PSUM"))
    psA1 = ctx.enter_context(tc.tile_pool(name="psA1", bufs=2, space="PSUM"))
    psY = ctx.enter_context(tc.tile_pool(name="psY", bufs=2, space="PSUM"))

    # ---- packed consts [128, *] ----
    W_CWAB, W_W2T, W_W1T, W_CWC = 512, 396, 512, 512
    Wc = W_CWAB + W_W2T + W_W1T + W_CWC
    cst = const.tile([128, Wc], BF16)
    o = 0
    w1t = cst[0:96, o:o + W_W1T]; o += W_W1T
    w2w = cst[:, o:o + W_W2T]; o += W_W2T
    cwAB = cst[:, o:o + W_CWAB]; o += W_CWAB
    cwC = cst[0:64, o:o + W_CWC]; o += W_CWC

    if has_b1:
        b1c = const.tile([128, 4], FP32)
        nc.sync.dma_start(out=b1c[:], in_=ins["b1c"][:])
    if has_b2:
        b2r = const.tile([128, 196], BF16)
        nc.sync.dma_start(out=b2r[:], in_=ins["b2r"][:])

    axt = const.tile([64, G * 160], BF16)
    HX = 8 * 160
    nc.sync.dma_start(out=axt[:, 0:HX], in_=axt_d[:, 0:HX])
    nc.gpsimd.dma_start(out=cst[:, 0:908], in_=cst_d[:, 0:908])
    nc.scalar.dma_start(out=axt[:, HX:4 * HX], in_=axt_d[:, HX:4 * HX])
    nc.sync.dma_start(out=cst[:, 908:], in_=cst_d[:, 908:])
    nc.scalar.dma_start(out=axt[:, 4 * HX:], in_=axt_d[:, 4 * HX:])

    def a_g(j, g):
        base = 160 * (2 * j + g) + 96
        return axt[:, base:base + 64]

    uzps = {}
    a1ts = {}
    z2ws = {}
    z2los = {}
    hhs = {}

    def s_B1(j):
        # u = X^T A~  [96, (g, 64)]; a1t = W1 u  [128 (h), (c, m)]
        UZP = psU.tile([128, 423], FP32, tag="uzp")
        uzps[j] = UZP
        uP = UZP[0:96, 0:128]
        for g in range(2):
            xts = axt[:, 160 * (2 * j + g):160 * (2 * j + g) + 96]
            nc.tensor.matmul(uP[:, 64 * g:64 * (g + 1)], xts, a_g(j, g),
                             start=True, stop=True)
        u = sb.tile([96, 128], BF16, tag="u_sb", bufs=6)
        nc.scalar.activation(out=u[:], in_=uP, func=AF.Copy)
        a1tP = psA1.tile([128, 512], FP32, tag="a1t")
        a1ts[j] = a1tP
        for c in range(4):
            nc.tensor.matmul(a1tP[:, 128 * c:128 * (c + 1)],
                             w1t[:, 128 * c:128 * (c + 1)], u[:],
                             start=True, stop=True)

    h1ts = {}

    def s_relu(j):
        a1tP = a1ts.pop(j)
        h1t = sb.tile([128, 512], BF16, tag="h1t", bufs=6)
        h1ts[j] = (h1t, a1tP)
        if has_b1:
            for c in range(2):
                nc.scalar.activation(out=h1t[:, 128 * c:128 * (c + 1)],
                                     in_=a1tP[:, 128 * c:128 * (c + 1)],
                                     func=AF.Relu, bias=b1c[:, c:c + 1])
            for c in range(2, 4):
                nc.vector.tensor_scalar(
                    out=h1t[:, 128 * c:128 * (c + 1)],
                    in0=a1tP[:, 128 * c:128 * (c + 1)],
                    scalar1=b1c[:, c:c + 1], scalar2=0.0,
                    op0=OP.add, op1=OP.max)
        else:
            nc.scalar.activation(out=h1t[:], in_=a1tP[:], func=AF.Relu)

    def s_B2a(j):
        # z2 mms -> z2w evac
        UZP = uzps[j]
        z2P = UZP[:, 128:227]
        h1t, _ = h1ts.pop(j)
        for c in range(4):
            nc.tensor.matmul(z2P[:], h1t[:, 128 * c:128 * (c + 1)],
                             w2w[:, 99 * c:99 * (c + 1)],
                             start=(c == 0), stop=(c == 3))
        z2w = sb.tile([128, 99], BF16, tag="z2w", bufs=6)
        nc.vector.tensor_copy(out=z2w[:], in_=z2P[:])
        z2ws[j] = z2w
        z2lo = sb.tile([64, 99], BF16, tag="z2lo", bufs=6)
        nc.gpsimd.tensor_copy(out=z2lo[:], in_=z2w[64:128, :])
        z2los[j] = z2lo

    def s_B2b(j):
        # h2 into P (reads z2w/z2lo evacuated one stage earlier)
        UZP = uzps[j]
        P = UZP[:, 227:423]
        z2w = z2ws.pop(j)
        z2lo = z2los.pop(j)
        for g in range(2):
            rhs = z2w[0:64, :] if g == 0 else z2lo[:]
            base = 98 * g
            nc.tensor.matmul(P[0:64, base:base + 98], a_g(j, g),
                             rhs[:, 0:98], start=True, stop=True)
            nc.tensor.matmul(P[64:128, base:base + 98], a_g(j, g),
                             rhs[:, 1:99], start=True, stop=True,
                             tile_position=(0, 64))

    def s_HH(j):
        P = uzps.pop(j)[:, 227:423]
        HH = sb.tile([128, 196], BF16, tag="HH", bufs=6)
        if has_b2:
            nc.vector.tensor_tensor(out=HH[:], in0=P[:], in1=b2r[:],
                                    op=OP.add)
        else:
            nc.scalar.activation(out=HH[:], in_=P[:], func=AF.Copy)
        hhs[j] = HH

    def s_B3(j, ysbG):
        # conv -> y evac
        HH = hhs.pop(j)
        HH_A = HH[:].rearrange("p (g w) -> p g w", w=98)[:, :, 0:96]
        HH_B = HH[0:64, :].rearrange("p (g w) -> p g w", w=98)[:, :, 2:98]
        # ysbG layout: [p, (oc4, pr2, j192)] so the group DMA balances to 3D
        pr = j % 2
        ysbG_v = ysbG[:].rearrange("p (oc pr j) -> p oc pr j", oc=4, pr=2)
        for half in range(2):
            yT = psY.tile([128, 384], FP32, tag="y")
            for k in range(2):
                oc = 2 * half + k
                out_sl = yT[:, 192 * k:192 * (k + 1)]
                nc.tensor.matmul(out_sl, cwAB[:, 128 * oc:128 * (oc + 1)],
                                 HH_A, start=True, stop=False)
                nc.tensor.matmul(out_sl, cwC[:, 128 * oc:128 * (oc + 1)],
                                 HH_B, start=False, stop=True)
            dst = ysbG_v[:, 2 * half:2 * half + 2, pr, :]
            src = yT[:].rearrange("p (k j) -> p k j", k=2)
            nc.vector.tensor_copy(out=dst, in_=src)

    ysbGs = {}
    DF = 0      # pairs emitted fully unstaged to prime the pipeline
    for j in range(DF):
        if j % 2 == 0:
            ysbGs[j // 2] = sb.tile([128, 1536], BF16, tag="ysbG",
                                    bufs=6, name=f"ysbGp{j // 2}")
        s_B1(j)
        s_relu(j)
        s_B2a(j)
        s_B2b(j)
        s_HH(j)
        s_B3(j, ysbGs[j // 2])
        if j % 2 == 1:
            q = j // 2
            dstp = y_d[:, :, 2 * q:2 * q + 2, :] \
                .rearrange("oc p pr j2 -> p oc pr j2")
            nc.sync.dma_start(
                out=dstp,
                in_=ysbGs.pop(q)[:].rearrange(
                    "p (oc pr j2) -> p oc pr j2", oc=4, pr=2))
    for i in range(DF, NPAIR + 4):
        if DF <= i - 3 and i - 3 < NPAIR:
            s_HH(i - 3)
        if DF <= i - 1 and i - 1 < NPAIR:
            s_relu(i - 1)
        if i < NPAIR:
            s_B1(i)
        if DF <= i - 2 and i - 2 < NPAIR:
            s_B2b(i - 2)
        if DF <= i - 1 and i - 1 < NPAIR:
            s_B2a(i - 1)
        j = i - 4
        if DF <= j:
            if j % 2 == 0:
                ysbGs[j // 2] = sb.tile([128, 1536], BF16, tag="ysbG",
                                        bufs=6, name=f"ysbG{j // 2}")
            s_B3(j, ysbGs[j // 2])
            if j == NPAIR - 2:
                # ship pair 30 alone so the final DMA is small
                q = j // 2
                dst = y_d[:, :, j:j + 1, :] \
                    .rearrange("oc p pr j2 -> p oc pr j2")
                nc.sync.dma_start(
                    out=dst,
                    in_=ysbGs[q][:].rearrange(
                        "p (oc pr j2) -> p oc pr j2", oc=4, pr=2)[:, :, 0:1, :])
            elif j == NPAIR - 1:
                q = j // 2
                dst = y_d[:, :, j:j + 1, :] \
                    .rearrange("oc p pr j2 -> p oc pr j2")
                nc.sync.dma_start(
                    out=dst,
                    in_=ysbGs.pop(q)[:].rearrange(
                        "p (oc pr j2) -> p oc pr j2", oc=4, pr=2)[:, :, 1:2, :])
            elif j % 2 == 1:
                q = j // 2
                dst = y_d[:, :, 2 * q:2 * q + 2, :] \
                    .rearrange("oc p pr j2 -> p oc pr j2")
                nc.sync.dma_start(
                    out=dst,
                    in_=ysbGs.pop(q)[:].rearrange(
                        "p (oc pr j2) -> p oc pr j2", oc=4, pr=2))

    ctx.close()


# ---------------- host side ----------------

def _prep_consts(W1, b1, W2, b2, conv_w):
    bf = ml_dtypes.bfloat16
    cw = conv_w.astype(np.float32)          # [512, 64, 3]
    cw0 = cw[:, :, 0].T                     # [64, 512]
    cw1 = cw[:, :, 1].T
    cw2 = cw[:, :, 2].T
    cwAB = np.concatenate([cw0, cw1], axis=0).astype(bf)       # [128, 512]
    w2t4 = W2.T.reshape(4, 128, 96).transpose(1, 0, 2)       # [128, 4, 96]
    w2w4 = np.concatenate([w2t4[:, :, 95:96], w2t4,
                           w2t4[:, :, 0:2]], axis=2)          # [128, 4, 99]
    w2w = np.ascontiguousarray(w2w4.reshape(128, 396)).astype(bf)
    w1t = np.zeros((128, 512), bf)
    w1t[0:96, :] = np.ascontiguousarray(W1.T).astype(bf)
    cwC = np.zeros((128, 512), bf)
    cwC[0:64, :] = cw2.astype(bf)
    cst = np.concatenate([w1t, w2w, cwAB, cwC], axis=1)
    consts = dict(cst=np.ascontiguousarray(cst))
    has_b1 = bool(np.any(b1))
    has_b2 = bool(np.any(b2))
    if has_b1:
        consts["b1c"] = np.ascontiguousarray(
            b1.reshape(4, 128).T).astype(np.float32)
    if has_b2:
        pad = np.concatenate([b2[95:96], b2, b2[0:1]])        # [98]
        row = np.tile(pad, 2)                                  # [196]
        consts["b2r"] = np.ascontiguousarray(
            np.broadcast_to(row.astype(bf), (128, 196)))
    return consts, has_b1, has_b2


def _norm_adj(edge_index):
    """A~[g] = Dinv (C+I)^T Dinv per graph, [B, n_src, n_dst] f32."""
    b = edge_index.shape[0]
    src = edge_index[:, 0, :].astype(np.int64)      # [b, E]
    dst = edge_index[:, 1, :].astype(np.int64)
    flat = (np.arange(b)[:, None] * (N * N) + dst * N + src).ravel()
    C = np.bincount(flat, minlength=b * N * N).reshape(b, N, N)
    C = C.astype(np.float32) + np.eye(N, dtype=np.float32)[None]
    deg = C.sum(axis=2)                              # in-degree + 1
    dinv = 1.0 / np.sqrt(deg)                        # deg >= 1 always
    # atil[s, d] = dinv[s] * C[d, s] * dinv[d]
    return dinv[:, :, None] * C.transpose(0, 2, 1) * dinv[:, None, :]


_NC_CACHE = {}


def _get_nc(has_b1, has_b2):
    key = (has_b1, has_b2)
    if key in _NC_CACHE:
        return _NC_CACHE[key]
    nc = bacc.Bacc("TRN2", target_bir_lowering=False, debug=False)
    Wc = 512 + 396 + 512 + 512
    ins = {
        "axt": nc.dram_tensor("axt", [64, G * 160], BF16,
                              kind="ExternalInput").ap(),
        "cst": nc.dram_tensor("cst", [128, Wc], BF16,
                              kind="ExternalInput").ap(),
    }
    if has_b1:
        ins["b1c"] = nc.dram_tensor("b1c", [128, 4], FP32,
                                    kind="ExternalInput").ap()
    if has_b2:
        ins["b2r"] = nc.dram_tensor("b2r", [128, 196], BF16,
                                    kind="ExternalInput").ap()
    outs = {
        "y": nc.dram_tensor("y", [4, 128, NPAIR, 192], BF16,
                            kind="ExternalOutput").ap(),
    }
    with tile.TileContext(nc) as tc:
        build_gcn_kernel(tc, outs, ins, has_b1, has_b2)
    nc.compile()
    _NC_CACHE[key] = nc
    return nc


def kernel(x, edge_index, W1, b1, W2, b2, conv_w, _trace=False):
    x = np.asarray(x, dtype=np.float32)
    edge_index = np.asarray(edge_index)
    consts, has_b1, has_b2 = _prep_consts(
        np.asarray(W1, np.float32), np.asarray(b1, np.float32),
        np.asarray(W2, np.float32), np.asarray(b2, np.float32),
        np.asarray(conv_w, np.float32))
    nc = _get_nc(has_b1, has_b2)

    bf = ml_dtypes.bfloat16
    atil = _norm_adj(edge_index)                      # [B, 64, 64] f32
    in_maps = []
    for core in range(N_CORES):
        sl = slice(core * G, (core + 1) * G)
        xs = x[sl]                                    # [G, 96, 64]
        xtc = xs.transpose(2, 0, 1)                   # [64, G, 96]
        atc = atil[sl].transpose(1, 0, 2)             # [64, G, 64]
        axt = np.ascontiguousarray(
            np.concatenate([xtc, atc], axis=2).reshape(64, G * 160)
        ).astype(bf)
        m = dict(consts)
        m["axt"] = axt
        in_maps.append(m)

    res = run_bass_kernel_spmd(nc, in_maps, core_ids=list(range(N_CORES)),
                               trace=_trace)
    out = np.empty((B, S, H), np.float32)
    for core in range(N_CORES):
        yT = res.results[core]["y"].astype(np.float32)  # [4, 128, 32, 192]
        yc = yT.reshape(4, 128, NPAIR, 2, 96).transpose(2, 3, 4, 0, 1) \
               .reshape(G, 96, 512)
        out[core * G:(core + 1) * G] = yc
    if _trace:
        kernel.last_results = res
    return out


# revision 56
# speedup vs baseline: 1.0261x; 1.0261x over previous
"""Trainium2 Bass kernel for batched GCN (2x GCNConv + circular Conv1d).

Math per graph (N=64 nodes, S=96 feats, H=512 hidden, E=512 edges):
    A~       = Dinv (C+I)^T Dinv    (normalized adjacency, transposed)
    u        = X^T A~               ( = (A X)^T,  X = x.T [n, s])
    a1t      = W1 u                 (h on partitions, = (A X W1^T)^T)
    h1t      = relu(a1t)
    z2       = h1 W2^T              (via 4 h-chunk accumulation)
    h2       = A z2   (written shifted+duplicated into P for the conv)
    y        = circular_conv1d(h2, conv_w), emitted transposed [o, (g,l)]

The normalized adjacency A~ is built on the host from edge_index
(vectorized bincount + outer scaling -- standard GNN graph
preprocessing) and shipped per graph as a compact [64, 64] bf16 tile;
all model FLOPs (4 matmul stages + conv) run on device.

Device pipeline: 64 graphs = 32 pairs, pair in partition halves.
  Per pair: u (2 mm) -> a1t (4 mm) -> relu -> z2 (4 mm, emits the
  wrap-padded z2w directly via duplicated W2 columns) -> h2 (4 mm,
  shift-duplicated into P) -> conv (8 mm) -> y evac -> batched DMA.
  Emission is software-pipelined over 4 stage-lags (B1 | relu+B2a |
  B2b | HH+B3) so every engine's in-order queue sees dep-ready work;
  psum = u+z2w+P ring 4 | a1t ring 2 | y ring 2 = 8 banks; psum
  evacuations are balanced Act{u, relu, HH} vs DVE{z2w, yA, yB}
  (min-max over engine-specific fixed+per-col costs), sbuf-only copies
  on GpSimd; inputs split across SP/Act/GpSimd DMA queues; out DMA
  batched 2 pairs/instruction.
"""

import numpy as np
import ml_dtypes

import concourse.bacc as bacc
import concourse.mybir as mybir
import concourse.tile as tile
from concourse.bass_utils import run_bass_kernel_spmd

BF16 = mybir.dt.bfloat16
FP32 = mybir.dt.float32
AF = mybir.ActivationFunctionType
OP = mybir.AluOpType

N_CORES = 8
B, S, N, H, E = 512, 96, 64, 512, 512
G = B // N_CORES          # graphs per core (64)
NPAIR = G // 2            # 32


def build_gcn_kernel(tc, outs, ins, has_b1=False, has_b2=False):
    nc = tc.nc

    axt_d = ins["axt"]      # [64, G*160] bf16  per graph [x^T | A~] on parts 0:64
    cst_d = ins["cst"]      # [128, Wc] bf16 packed consts
    y_d = outs["y"]         # [4, 128, 32, 192] bf16

    from contextlib import ExitStack
    ctx = ExitStack()
    const = ctx.enter_context(tc.tile_pool(name="const", bufs=1))
    sb = ctx.enter_context(tc.tile_pool(name="sb", bufs=1))
    psU = ctx.enter_context(tc.tile_pool(name="psU", bufs=4, space="PSUM"))
    psA1 = ctx.enter_context(tc.tile_pool(name="psA1", bufs=2, space="PSUM"))
    psY = ctx.enter_context(tc.tile_pool(name="psY", bufs=2, space="PSUM"))

    # ---- packed consts [128, *] ----
    W_CWAB, W_W2T, W_W1T, W_CWC = 512, 396, 512, 512
    Wc = W_CWAB + W_W2T + W_W1T + W_CWC
    cst = const.tile([128, Wc], BF16)
    o = 0
    w1t = cst[0:96, o:o + W_W1T]; o += W_W1T
    w2w = cst[:, o:o + W_W2T]; o += W_W2T
    cwAB = cst[:, o:o + W_CWAB]; o += W_CWAB
    cwC = cst[0:64, o:o + W_CWC]; o += W_CWC

    if has_b1:
        b1c = const.tile([128, 4], FP32)
        nc.sync.dma_start(out=b1c[:], in_=ins["b1c"][:])
    if has_b2:
        b2r = const.tile([128, 196], BF16)
        nc.sync.dma_start(out=b2r[:], in_=ins["b2r"][:])

    axt = const.tile([64, G * 160], BF16)
    HX = 8 * 160
    nc.sync.dma_start(out=axt[:, 0:HX], in_=axt_d[:, 0:HX])
    nc.gpsimd.dma_start(out=cst[:, 0:908], in_=cst_d[:, 0:908])
    nc.scalar.dma_start(out=axt[:, HX:4 * HX], in_=axt_d[:, HX:4 * HX])
    nc.sync.dma_start(out=cst[:, 908:], in_=cst_d[:, 908:])
    nc.scalar.dma_start(out=axt[:, 4 * HX:], in_=axt_d[:, 4 * HX:])

    def a_g(j, g):
        base = 160 * (2 * j + g) + 96
        return axt[:, base:base + 64]

    uzps = {}
    a1ts = {}
    z2ws = {}
    z2los = {}
    hhs = {}

    def s_B1(j):
        # u = X^T A~  [96, (g, 64)]; a1t = W1 u  [128 (h), (c, m)]
        UZP = psU.tile([128, 423], FP32, tag="uzp")
        uzps[j] = UZP
        uP = UZP[0:96, 0:128]
        for g in range(2):
            xts = axt[:, 160 * (2 * j + g):160 * (2 * j + g) + 96]
            nc.tensor.matmul(uP[:, 64 * g:64 * (g + 1)], xts, a_g(j, g),
                             start=True, stop=True)
        u = sb.tile([96, 128], BF16, tag="u_sb", bufs=6)
        nc.scalar.activation(out=u[:], in_=uP, func=AF.Copy)
        a1tP = psA1.tile([128, 512], FP32, tag="a1t")
        a1ts[j] = a1tP
        for c in range(4):
            nc.tensor.matmul(a1tP[:, 128 * c:128 * (c + 1)],
                             w1t[:, 128 * c:128 * (c + 1)], u[:],
                             start=True, stop=True)

    h1ts = {}

    def s_relu(j):
        a1tP = a1ts.pop(j)
        h1t = sb.tile([128, 512], BF16, tag="h1t", bufs=6)
        h1ts[j] = (h1t, a1tP)
        if has_b1:
            for c in range(2):
                nc.scalar.activation(out=h1t[:, 128 * c:128 * (c + 1)],
                                     in_=a1tP[:, 128 * c:128 * (c + 1)],
                                     func=AF.Relu, bias=b1c[:, c:c + 1])
            for c in range(2, 4):
                nc.vector.tensor_scalar(
                    out=h1t[:, 128 * c:128 * (c + 1)],
                    in0=a1tP[:, 128 * c:128 * (c + 1)],
                    scalar1=b1c[:, c:c + 1], scalar2=0.0,
                    op0=OP.add, op1=OP.max)
        else:
            nc.scalar.activation(out=h1t[:], in_=a1tP[:], func=AF.Relu)

    def s_B2a(j):
        # z2 mms -> z2w evac
        UZP = uzps[j]
        z2P = UZP[:, 128:227]
        h1t, _ = h1ts.pop(j)
        for c in range(4):
            nc.tensor.matmul(z2P[:], h1t[:, 128 * c:128 * (c + 1)],
                             w2w[:, 99 * c:99 * (c + 1)],
                             start=(c == 0), stop=(c == 3))
        z2w = sb.tile([128, 99], BF16, tag="z2w", bufs=6)
        nc.vector.tensor_copy(out=z2w[:], in_=z2P[:])
        z2ws[j] = z2w
        z2lo = sb.tile([64, 99], BF16, tag="z2lo", bufs=6)
        nc.gpsimd.tensor_copy(out=z2lo[:], in_=z2w[64:128, :])
        z2los[j] = z2lo

    def s_B2b(j):
        # h2 into P (reads z2w/z2lo evacuated one stage earlier)
        UZP = uzps[j]
        P = UZP[:, 227:423]
        z2w = z2ws.pop(j)
        z2lo = z2los.pop(j)
        for g in range(2):
            rhs = z2w[0:64, :] if g == 0 else z2lo[:]
            base = 98 * g
            nc.tensor.matmul(P[0:64, base:base + 98], a_g(j, g),
                             rhs[:, 0:98], start=True, stop=True)
            nc.tensor.matmul(P[64:128, base:base + 98], a_g(j, g),
                             rhs[:, 1:99], start=True, stop=True,
                             tile_position=(0, 64))

    def s_HH(j):
        P = uzps.pop(j)[:, 227:423]
        HH = sb.tile([128, 196], BF16, tag="HH", bufs=6)
        if has_b2:
            nc.vector.tensor_tensor(out=HH[:], in0=P[:], in1=b2r[:],
                                    op=OP.add)
        else:
            nc.scalar.activation(out=HH[:], in_=P[:], func=AF.Copy)
        hhs[j] = HH

    def s_B3(j, ysbG):
        # conv -> y evac
        HH = hhs.pop(j)
        HH_A = HH[:].rearrange("p (g w) -> p g w", w=98)[:, :, 0:96]
        HH_B = HH[0:64, :].rearrange("p (g w) -> p g w", w=98)[:, :, 2:98]
        # ysbG layout: [p, (oc4, pr2, j192)] so the group DMA balances to 3D
        pr = j % 2
        ysbG_v = ysbG[:].rearrange("p (oc pr j) -> p oc pr j", oc=4, pr=2)
        for half in range(2):
            yT = psY.tile([128, 384], FP32, tag="y")
            for k in range(2):
                oc = 2 * half + k
                out_sl = yT[:, 192 * k:192 * (k + 1)]
                nc.tensor.matmul(out_sl, cwAB[:, 128 * oc:128 * (oc + 1)],
                                 HH_A, start=True, stop=False)
                nc.tensor.matmul(out_sl, cwC[:, 128 * oc:128 * (oc + 1)],
                                 HH_B, start=False, stop=True)
            dst = ysbG_v[:, 2 * half:2 * half + 2, pr, :]
            src = yT[:].rearrange("p (k j) -> p k j", k=2)
            nc.vector.tensor_copy(out=dst, in_=src)

    ysbGs = {}
    DF = 0      # pairs emitted fully unstaged to prime the pipeline
    for j in range(DF):
        if j % 2 == 0:
            ysbGs[j // 2] = sb.tile([128, 1536], BF16, tag="ysbG",
                                    bufs=6, name=f"ysbGp{j // 2}")
        s_B1(j)
        s_relu(j)
        s_B2a(j)
        s_B2b(j)
        s_HH(j)
        s_B3(j, ysbGs[j // 2])
        if j % 2 == 1:
            q = j // 2
            dstp = y_d[:, :, 2 * q:2 * q + 2, :] \
                .rearrange("oc p pr j2 -> p oc pr j2")
            nc.sync.dma_start(
                out=dstp,
                in_=ysbGs.pop(q)[:].rearrange(
                    "p (oc pr j2) -> p oc pr j2", oc=4, pr=2))
    for i in range(DF, NPAIR + 3):
        if DF <= i - 3 and i - 3 < NPAIR:
            s_HH(i - 3)
        if DF <= i - 1 and i - 1 < NPAIR:
            s_relu(i - 1)
        if i < NPAIR:
            s_B1(i)
        if DF <= i - 2 and i - 2 < NPAIR:
            s_B2b(i - 2)
        if DF <= i - 1 and i - 1 < NPAIR:
            s_B2a(i - 1)
        j = i - 3
        if DF <= j:
            if j % 2 == 0:
                ysbGs[j // 2] = sb.tile([128, 1536], BF16, tag="ysbG",
                                        bufs=6, name=f"ysbG{j // 2}")
            s_B3(j, ysbGs[j // 2])
            if j == NPAIR - 2:
                # ship pair 30 alone so the final DMA is small
                q = j // 2
                dst = y_d[:, :, j:j + 1, :] \
                    .rearrange("oc p pr j2 -> p oc pr j2")
                nc.sync.dma_start(
                    out=dst,
                    in_=ysbGs[q][:].rearrange(
                        "p (oc pr j2) -> p oc pr j2", oc=4, pr=2)[:, :, 0:1, :])
            elif j == NPAIR - 1:
                q = j // 2
                dst = y_d[:, :, j:j + 1, :] \
                    .rearrange("oc p pr j2 -> p oc pr j2")
                nc.sync.dma_start(
                    out=dst,
                    in_=ysbGs.pop(q)[:].rearrange(
                        "p (oc pr j2) -> p oc pr j2", oc=4, pr=2)[:, :, 1:2, :])
            elif j % 2 == 1:
                q = j // 2
                dst = y_d[:, :, 2 * q:2 * q + 2, :] \
                    .rearrange("oc p pr j2 -> p oc pr j2")
                nc.sync.dma_start(
                    out=dst,
                    in_=ysbGs.pop(q)[:].rearrange(
                        "p (oc pr j2) -> p oc pr j2", oc=4, pr=2))

    ctx.close()


# ---------------- host side ----------------

def _prep_consts(W1, b1, W2, b2, conv_w):
    bf = ml_dtypes.bfloat16
    cw = conv_w.astype(np.float32)          # [512, 64, 3]
    cw0 = cw[:, :, 0].T                     # [64, 512]
    cw1 = cw[:, :, 1].T
    cw2 = cw[:, :, 2].T
    cwAB = np.concatenate([cw0, cw1], axis=0).astype(bf)       # [128, 512]
    w2t4 = W2.T.reshape(4, 128, 96).transpose(1, 0, 2)       # [128, 4, 96]
    w2w4 = np.concatenate([w2t4[:, :, 95:96], w2t4,
                           w2t4[:, :, 0:2]], axis=2)          # [128, 4, 99]
    w2w = np.ascontiguousarray(w2w4.reshape(128, 396)).astype(bf)
    w1t = np.zeros((128, 512), bf)
    w1t[0:96, :] = np.ascontiguousarray(W1.T).astype(bf)
    cwC = np.zeros((128, 512), bf)
    cwC[0:64, :] = cw2.astype(bf)
    cst = np.concatenate([w1t, w2w, cwAB, cwC], axis=1)
    consts = dict(cst=np.ascontiguousarray(cst))
    has_b1 = bool(np.any(b1))
    has_b2 = bool(np.any(b2))
    if has_b1:
        consts["b1c"] = np.ascontiguousarray(
            b1.reshape(4, 128).T).astype(np.float32)
    if has_b2:
        pad = np.concatenate([b2[95:96], b2, b2[0:1]])        # [98]
        row = np.tile(pad, 2)                                  # [196]
        consts["b2r"] = np.ascontiguousarray(
            np.broadcast_to(row.astype(bf), (128, 196)))
    return consts, has_b1, has_b2


def _norm_adj(edge_index):
    """A~[g] = Dinv (C+I)^T Dinv per graph, [B, n_src, n_dst] f32."""
    b = edge_index.shape[0]
    src = edge_index[:, 0, :].astype(np.int64)      # [b, E]
    dst = edge_index[:, 1, :].astype(np.int64)
    flat = (np.arange(b)[:, None] * (N * N) + dst * N + src).ravel()
    C = np.bincount(flat, minlength=b * N * N).reshape(b, N, N)
    C = C.astype(np.float32) + np.eye(N, dtype=np.float32)[None]
    deg = C.sum(axis=2)                              # in-degree + 1
    dinv = 1.0 / np.sqrt(deg)                        # deg >= 1 always
    # atil[s, d] = dinv[s] * C[d, s] * dinv[d]
    return dinv[:, :, None] * C.transpose(0, 2, 1) * dinv[:, None, :]


_NC_CACHE = {}


def _get_nc(has_b1, has_b2):
    key = (has_b1, has_b2)
    if key in _NC_CACHE:
        return _NC_CACHE[key]
    nc = bacc.Bacc("TRN2", target_bir_lowering=False, debug=False)
    Wc = 512 + 396 + 512 + 512
    ins = {
        "axt": nc.dram_tensor("axt", [64, G * 160], BF16,
                              kind="ExternalInput").ap(),
        "cst": nc.dram_tensor("cst", [128, Wc], BF16,
                              kind="ExternalInput").ap(),
    }
    if has_b1:
        ins["b1c"] = nc.dram_tensor("b1c", [128, 4], FP32,
                                    kind="ExternalInput").ap()
    if has_b2:
        ins["b2r"] = nc.dram_tensor("b2r", [128, 196], BF16,
                                    kind="ExternalInput").ap()
    outs = {
        "y": nc.dram_tensor("y", [4, 128, NPAIR, 192], BF16,
                            kind="ExternalOutput").ap(),
    }
    with tile.TileContext(nc) as tc:
        build_gcn_kernel(tc, outs, ins, has_b1, has_b2)
    nc.compile()
    _NC_CACHE[key] = nc
    return nc


def kernel(x, edge_index, W1, b1, W2, b2, conv_w, _trace=False):
    x = np.asarray(x, dtype=np.float32)
    edge_index = np.asarray(edge_index)
    consts, has_b1, has_b2 = _prep_consts(
        np.asarray(W1, np.float32), np.asarray(b1, np.float32),
        np.asarray(W2, np.float32), np.asarray(b2, np.float32),
        np.asarray(conv_w, np.float32))
    nc = _get_nc(has_b1, has_b2)

    bf = ml_dtypes.bfloat16
    atil = _norm_adj(edge_index)                      # [B, 64, 64] f32
    in_maps = []
    for core in range(N_CORES):
        sl = slice(core * G, (core + 1) * G)
        xs = x[sl]                                    # [G, 96, 64]
        xtc = xs.transpose(2, 0, 1)                   # [64, G, 96]
        atc = atil[sl].transpose(1, 0, 2)             # [64, G, 64]
        axt = np.ascontiguousarray(
            np.concatenate([xtc, atc], axis=2).reshape(64, G * 160)
        ).astype(bf)
        m = dict(consts)
        m["axt"] = axt
        in_maps.append(m)

    res = run_bass_kernel_spmd(nc, in_maps, core_ids=list(range(N_CORES)),
                               trace=_trace)
    out = np.empty((B, S, H), np.float32)
    for core in range(N_CORES):
        yT = res.results[core]["y"].astype(np.float32)  # [4, 128, 32, 192]
        yc = yT.reshape(4, 128, NPAIR, 2, 96).transpose(2, 3, 4, 0, 1) \
               .reshape(G, 96, 512)
        out[core * G:(core + 1) * G] = yc
    if _trace:
        kernel.last_results = res
    return out
